# revision 14
# baseline (speedup 1.0000x reference)
"""Trainium2 Bass kernel for nn_LocalLocalContrastiveLoss.

Math (see reference): z = z_t.reshape(N=4096, D=256); logits row i =
[sim(i, ·) with self masked, z@memQ.T] / T; lse_i = logsumexp(row);
per_pair_i = lse_i - sim(i, i+1)/T; loss = mean over valid anchors
(i % L != L-1), n_pairs = 4080.  va_values is unused (faithful to ref).

Distribution: 8 cores, each handles 512 anchors (4 blocks of 128).
Negatives (all of z + memory queue) are replicated. To keep one
core-agnostic NEFF, each core's copy of z^T is ROTATED so its own 512
anchor columns come first; then the self-diagonal sits at fixed block
positions identical on every core.

v3 design (after trace analysis of v2):
- bf16 matmul inputs (host-converted): 1 cyc/row on PE, half DMA.
- The whole 20480-col rhs stays RESIDENT in SBUF (40 bf16 tiles of
  [128,1024]; ~80KB/partition) - DMA'd once, reused by all 4 blocks.
- 1024-wide PSUM regions x 4 in flight (the full 8 banks). v2 used
  2x2048 which serialized the MM->reduce_max->exp chain (each engine
  only ~53% busy); 4 regions let DVE/ACT/PE run concurrently.
- Per (chunk, block): 4 matmuls (2 K-halves x 2x512) -> DVE reduce_max
  (negated) -> ACT exp(bias=-max) with accum_out -> per-chunk sums.
- pos-sims (z_i . z_{i+1}) and the final logsumexp merge + masked mean
  are done on host in fp64 (tiny: ~4096x256 MACs + 4096x80 merges).
"""

import sys
from contextlib import ExitStack

import ml_dtypes
import numpy as np

sys.path.insert(0, "/opt/trn_rl_repo")

import concourse.bass as bass  # noqa: E402
import concourse.bacc as bacc  # noqa: E402
import concourse.tile as tile  # noqa: E402
from concourse import mybir  # noqa: E402
from concourse.bass_utils import run_bass_kernel_spmd  # noqa: E402

B, L, D = 16, 256, 256
N = B * L            # 4096 anchors
K = 16384            # memory queue
INV_T = 1.0 / 0.07
NCORES = 8
APC = N // NCORES    # anchors per core = 512
NB = APC // 128      # anchor blocks per core = 4
CH = 1024            # chunk width (2 PSUM banks)
NCOLS = N + K        # 20480
NCH = NCOLS // CH    # 20 chunks (4 from z, 16 from memq)
SUB = 512            # matmul moving free dim (one PSUM bank)
F32 = mybir.dt.float32
BF16 = mybir.dt.bfloat16


def _build_nc() -> bass.Bass:
    nc = bacc.Bacc("TRN2", target_bir_lowering=False, debug=False)

    # K-halves share the middle dim so each chunk is ONE DMA (host lays
    # the data out as [128 partitions, 2 K-halves, cols]).
    anch = nc.dram_tensor("anch", [128, 2, APC], BF16, kind="ExternalInput")
    zrot = nc.dram_tensor("zrot", [128, 2, N], BF16, kind="ExternalInput")
    memq = nc.dram_tensor("memq", [128, 2, K], BF16, kind="ExternalInput")
    eyen = nc.dram_tensor("eyen", [128, 128], F32, kind="ExternalInput")
    stats_out = nc.dram_tensor(
        "stats_out", [128, 2, NB * NCH], F32, kind="ExternalOutput"
    )

    with tile.TileContext(nc) as tc, ExitStack() as ctx:
        consts = ctx.enter_context(tc.tile_pool(name="consts", bufs=1))
        psum = ctx.enter_context(tc.tile_pool(name="psum", bufs=4, space="PSUM"))
        stats = ctx.enter_context(tc.tile_pool(name="stats", bufs=1))

        # Stationary anchors (lhsT) + self-mask constant.
        anch_sb = consts.tile([128, 2, APC], BF16, tag="anch", name="anch_sb")
        nc.sync.dma_start(anch_sb[:], anch[:])

        # Whole rhs resident in SBUF: 20 chunks x [128, 2, CH], DMA'd once.
        # Startup: each DMA issue costs ~0.7us on its queue, so spread the
        # first chunks across engine queues (they're idle before compute)
        # and split chunks 0-1 in half so the first matmuls start sooner.
        rt = [consts.tile([128, 2, CH], BF16, tag=f"rt{c}", name=f"rt{c}")
              for c in range(NCH)]
        eyen_sb = consts.tile([128, 128], F32, tag="eyen", name="eyen_sb")

        def _src(c, lo, hi):
            if c < N // CH:
                return zrot[:, :, c * CH + lo:c * CH + hi]
            base = (c - N // CH) * CH
            return memq[:, :, base + lo:base + hi]

        nc.sync.dma_start(rt[0][:, :, 0:SUB], _src(0, 0, SUB))
        nc.scalar.dma_start(rt[0][:, :, SUB:CH], _src(0, SUB, CH))
        nc.gpsimd.dma_start(eyen_sb[:], eyen[:])
        nc.sync.dma_start(rt[1][:, :, 0:SUB], _src(1, 0, SUB))
        nc.scalar.dma_start(rt[1][:, :, SUB:CH], _src(1, SUB, CH))
        nc.gpsimd.dma_start(rt[2][:], _src(2, 0, CH))
        nc.scalar.dma_start(rt[3][:], _src(3, 0, CH))
        for c in range(4, NCH):
            nc.sync.dma_start(rt[c][:], _src(c, 0, CH))

        # Warm-up matmuls on the already-landed anchors: keeps PE busy
        # while rt0 streams in, so the HAM clock gate opens (1.2->2.4GHz)
        # before the real matmul stream begins.  Output is discarded.
        warm = psum.tile([128, CH], F32, tag="pt", name="warm")
        for w in range(4):
            nc.tensor.matmul(
                warm[:, 0:SUB], anch_sb[:, 0, 0:128], anch_sb[:, 0, 0:SUB],
                start=True, stop=True,
            )

        # stats[:, 0, bc] = negated chunk max; stats[:, 1, bc] = exp-sum
        st = stats.tile([128, 2, NB * NCH], F32, tag="st", name="st")

        for c in range(NCH):
            for b in range(NB):
                pt = psum.tile([128, CH], F32, tag="pt", name="pt")
                for k in range(2):
                    lhsT = anch_sb[:, k, b * 128:(b + 1) * 128]
                    for s in range(CH // SUB):
                        nc.tensor.matmul(
                            pt[:, s * SUB:(s + 1) * SUB],
                            lhsT,
                            rt[c][:, k, s * SUB:(s + 1) * SUB],
                            start=(k == 0),
                            stop=(k == 1),
                        )
                bc = b * NCH + c
                if c == 0:
                    # mask self-sim on the block diagonal: -= 1e30 * eye
                    diag = pt[:, b * 128:(b + 1) * 128]
                    nc.vector.tensor_sub(diag, diag, eyen_sb[:])
                nc.vector.reduce_max(
                    out=st[:, 0, bc:bc + 1], in_=pt[:], axis=mybir.AxisListType.X,
                    negate=True,
                )
                nc.scalar.activation(
                    out=pt[:], in_=pt[:], func=mybir.ActivationFunctionType.Exp,
                    bias=st[:, 0, bc:bc + 1], scale=1.0,
                    accum_out=st[:, 1, bc:bc + 1],
                )

        nc.sync.dma_start(stats_out[:], st[:])

    nc.compile()
    return nc


_NC_CACHE = None


def _get_nc():
    global _NC_CACHE
    if _NC_CACHE is None:
        _NC_CACHE = _build_nc()
    return _NC_CACHE


def _k_mid(a: np.ndarray) -> np.ndarray:
    # [D, cols] -> [128 partitions, 2 K-halves, cols]
    return np.ascontiguousarray(a.reshape(2, 128, a.shape[1]).transpose(1, 0, 2))


def make_in_maps(z_t: np.ndarray, memory_queue: np.ndarray):
    z = np.ascontiguousarray(z_t.reshape(N, D)).astype(np.float32)
    zT = np.ascontiguousarray(z.T)                      # [D, N]
    memT = np.ascontiguousarray(memory_queue.astype(np.float32).T)  # [D, K]
    memT = _k_mid(memT.astype(ml_dtypes.bfloat16))
    eyen = (np.eye(128, dtype=np.float32) * 1e30)

    in_maps = []
    for r in range(NCORES):
        zr = np.roll(zT, -APC * r, axis=1)              # own cols first
        anch = (zr[:, :APC] * np.float32(INV_T)).astype(ml_dtypes.bfloat16)
        in_maps.append({
            "anch": _k_mid(anch),
            "zrot": _k_mid(zr.astype(ml_dtypes.bfloat16)),
            "memq": memT,
            "eyen": eyen,
        })
    return in_maps


def combine_outputs(results, z: np.ndarray) -> np.ndarray:
    # results[r]["stats_out"]: [128, 2, NB*NCH]; anchor g = 512r+128b+p.
    # lse = M + log(sum_c S_c * exp(m_c - M)),  m_c = -nm_c, M = max_c m_c
    lse_all = np.empty(N, dtype=np.float64)
    for r in range(NCORES):
        stats = np.asarray(results[r]["stats_out"], dtype=np.float64)
        m = -stats[:, 0, :].reshape(128, NB, NCH)
        S = stats[:, 1, :].reshape(128, NB, NCH)
        M = m.max(axis=2)                                        # [128, NB]
        lse = M + np.log(np.sum(S * np.exp(m - M[:, :, None]), axis=2))
        for b in range(NB):
            g0 = APC * r + 128 * b
            lse_all[g0:g0 + 128] = lse[:, b]
    z64 = z.astype(np.float64)
    pos = np.einsum("ij,ij->i", z64[:-1], z64[1:]) * INV_T       # [N-1]
    pp = lse_all[:N - 1] - pos
    valid = (np.arange(N - 1) % L) != (L - 1)
    loss = pp[valid].sum() / valid.sum()
    return np.float32(loss)


def kernel(z_t, va_values=None, memory_queue=None, _trace=False):
    nc = _get_nc()
    in_maps = make_in_maps(z_t, memory_queue)
    res = run_bass_kernel_spmd(
        nc, in_maps, core_ids=list(range(NCORES)), trace=_trace,
    )
    out = combine_outputs(res.results, z_t.reshape(N, D))
    if _trace:
        kernel.last_result = res
    return out


if __name__ == "__main__":
    rng = np.random.default_rng(0)
    z_t = rng.standard_normal((B, L, D), dtype=np.float32)
    mq = rng.standard_normal((K, D), dtype=np.float32)
    va = rng.random((B, L, 2), dtype=np.float32)
    loss = kernel(z_t, va, mq)
    print("device loss:", loss)
    # numpy reference check
    z = z_t.reshape(N, D).astype(np.float64)
    sim = (z @ z.T) * INV_T
    msim = (z @ mq.astype(np.float64).T) * INV_T
    np.fill_diagonal(sim, -np.inf)
    logits = np.concatenate([sim, msim], axis=1)
    m = logits.max(axis=1, keepdims=True)
    lse = np.log(np.exp(logits - m).sum(axis=1)) + m[:, 0]
    pos = np.array([(z[i] @ z[i + 1]) * INV_T for i in range(N - 1)])
    ppz = -pos + lse[:-1]
    vald = (np.arange(N - 1) % L) != (L - 1)
    ref = ppz[vald].sum() / vald.sum()
    print("numpy  loss:", ref, " rel err:", abs(loss - ref) / abs(ref))


# revision 15
# speedup vs baseline: 1.0516x; 1.0516x over previous
"""Trainium2 Bass kernel for nn_LocalLocalContrastiveLoss.

Math (see reference): z = z_t.reshape(N=4096, D=256); logits row i =
[sim(i, ·) with self masked, z@memQ.T] / T; lse_i = logsumexp(row);
per_pair_i = lse_i - sim(i, i+1)/T; loss = mean over valid anchors
(i % L != L-1), n_pairs = 4080.  va_values is unused (faithful to ref).

Distribution: 8 cores, each handles 512 anchors (4 blocks of 128).
Negatives (all of z + memory queue) are replicated. To keep one
core-agnostic NEFF, each core's copy of z^T is ROTATED so its own 512
anchor columns come first; then the self-diagonal sits at fixed block
positions identical on every core.

v3 design (after trace analysis of v2):
- bf16 matmul inputs (host-converted): 1 cyc/row on PE, half DMA.
- The whole 20480-col rhs stays RESIDENT in SBUF (40 bf16 tiles of
  [128,1024]; ~80KB/partition) - DMA'd once, reused by all 4 blocks.
- 1024-wide PSUM regions x 4 in flight (the full 8 banks). v2 used
  2x2048 which serialized the MM->reduce_max->exp chain (each engine
  only ~53% busy); 4 regions let DVE/ACT/PE run concurrently.
- Per (chunk, block): 4 matmuls (2 K-halves x 2x512) -> DVE reduce_max
  (negated) -> ACT exp(bias=-max) with accum_out -> per-chunk sums.
- pos-sims (z_i . z_{i+1}) and the final logsumexp merge + masked mean
  are done on host in fp64 (tiny: ~4096x256 MACs + 4096x80 merges).
"""

import sys
from contextlib import ExitStack

import ml_dtypes
import numpy as np

sys.path.insert(0, "/opt/trn_rl_repo")

import concourse.bass as bass  # noqa: E402
import concourse.bacc as bacc  # noqa: E402
import concourse.tile as tile  # noqa: E402
from concourse import mybir  # noqa: E402
from concourse.bass_utils import run_bass_kernel_spmd  # noqa: E402

B, L, D = 16, 256, 256
N = B * L            # 4096 anchors
K = 16384            # memory queue
INV_T = 1.0 / 0.07
NCORES = 8
APC = N // NCORES    # anchors per core = 512
NB = APC // 128      # anchor blocks per core = 4
CH = 1024            # chunk width (2 PSUM banks)
NCOLS = N + K        # 20480
NCH = NCOLS // CH    # 20 chunks (4 from z, 16 from memq)
SUB = 512            # matmul moving free dim (one PSUM bank)
F32 = mybir.dt.float32
BF16 = mybir.dt.bfloat16


def _build_nc() -> bass.Bass:
    nc = bacc.Bacc("TRN2", target_bir_lowering=False, debug=False)

    # K-halves share the middle dim so each chunk is ONE DMA (host lays
    # the data out as [128 partitions, 2 K-halves, cols]).
    anch = nc.dram_tensor("anch", [128, 2, APC], BF16, kind="ExternalInput")
    zrot = nc.dram_tensor("zrot", [128, 2, N], BF16, kind="ExternalInput")
    memq = nc.dram_tensor("memq", [128, 2, K], BF16, kind="ExternalInput")
    eyen = nc.dram_tensor("eyen", [128, 128], F32, kind="ExternalInput")
    stats_out = nc.dram_tensor(
        "stats_out", [128, 2, NB * NCH], F32, kind="ExternalOutput"
    )

    with tile.TileContext(nc) as tc, ExitStack() as ctx:
        consts = ctx.enter_context(tc.tile_pool(name="consts", bufs=1))
        psum = ctx.enter_context(tc.tile_pool(name="psum", bufs=4, space="PSUM"))
        stats = ctx.enter_context(tc.tile_pool(name="stats", bufs=1))

        # Stationary anchors (lhsT) + self-mask constant.
        anch_sb = consts.tile([128, 2, APC], BF16, tag="anch", name="anch_sb")
        nc.sync.dma_start(anch_sb[:], anch[:])

        # Whole rhs resident in SBUF: 20 chunks x [128, 2, CH], DMA'd once.
        # Startup: each DMA issue costs ~0.7us on its queue, so spread the
        # first chunks across engine queues (they're idle before compute)
        # and split chunks 0-1 in half so the first matmuls start sooner.
        rt = [consts.tile([128, 2, CH], BF16, tag=f"rt{c}", name=f"rt{c}")
              for c in range(NCH)]
        eyen_sb = consts.tile([128, 128], F32, tag="eyen", name="eyen_sb")

        def _src(c, lo, hi):
            if c < N // CH:
                return zrot[:, :, c * CH + lo:c * CH + hi]
            base = (c - N // CH) * CH
            return memq[:, :, base + lo:base + hi]

        # Single queue: transfers complete in issue order, so order by
        # when the compute stream first needs each tile.  rt0 is split so
        # the first matmuls (s=0, both K-halves) start ~1.6us earlier.
        nc.sync.dma_start(rt[0][:, :, 0:SUB], _src(0, 0, SUB))
        nc.sync.dma_start(rt[0][:, :, SUB:CH], _src(0, SUB, CH))
        nc.sync.dma_start(eyen_sb[:], eyen[:])
        nc.sync.dma_start(rt[1][:, :, 0:SUB], _src(1, 0, SUB))
        nc.sync.dma_start(rt[1][:, :, SUB:CH], _src(1, SUB, CH))
        for c in range(2, NCH):
            nc.sync.dma_start(rt[c][:], _src(c, 0, CH))

        # Warm-up matmuls on the already-landed anchors: keeps PE busy
        # while rt0 streams in, so the HAM clock gate opens (1.2->2.4GHz)
        # before the real matmul stream begins.  Output is discarded.
        warm = psum.tile([128, CH], F32, tag="pt", name="warm")
        for w in range(4):
            nc.tensor.matmul(
                warm[:, 0:SUB], anch_sb[:, 0, 0:128], anch_sb[:, 0, 0:SUB],
                start=True, stop=True,
            )

        # stats[:, 0, bc] = negated chunk max; stats[:, 1, bc] = exp-sum
        st = stats.tile([128, 2, NB * NCH], F32, tag="st", name="st")

        for c in range(NCH):
            for b in range(NB):
                pt = psum.tile([128, CH], F32, tag="pt", name="pt")
                for k in range(2):
                    lhsT = anch_sb[:, k, b * 128:(b + 1) * 128]
                    for s in range(CH // SUB):
                        nc.tensor.matmul(
                            pt[:, s * SUB:(s + 1) * SUB],
                            lhsT,
                            rt[c][:, k, s * SUB:(s + 1) * SUB],
                            start=(k == 0),
                            stop=(k == 1),
                        )
                bc = b * NCH + c
                if c == 0:
                    # mask self-sim on the block diagonal: -= 1e30 * eye
                    diag = pt[:, b * 128:(b + 1) * 128]
                    nc.vector.tensor_sub(diag, diag, eyen_sb[:])
                nc.vector.reduce_max(
                    out=st[:, 0, bc:bc + 1], in_=pt[:], axis=mybir.AxisListType.X,
                    negate=True,
                )
                nc.scalar.activation(
                    out=pt[:], in_=pt[:], func=mybir.ActivationFunctionType.Exp,
                    bias=st[:, 0, bc:bc + 1], scale=1.0,
                    accum_out=st[:, 1, bc:bc + 1],
                )

        nc.sync.dma_start(stats_out[:], st[:])

    nc.compile()
    return nc


_NC_CACHE = None


def _get_nc():
    global _NC_CACHE
    if _NC_CACHE is None:
        _NC_CACHE = _build_nc()
    return _NC_CACHE


def _k_mid(a: np.ndarray) -> np.ndarray:
    # [D, cols] -> [128 partitions, 2 K-halves, cols]
    return np.ascontiguousarray(a.reshape(2, 128, a.shape[1]).transpose(1, 0, 2))


def make_in_maps(z_t: np.ndarray, memory_queue: np.ndarray):
    z = np.ascontiguousarray(z_t.reshape(N, D)).astype(np.float32)
    zT = np.ascontiguousarray(z.T)                      # [D, N]
    memT = np.ascontiguousarray(memory_queue.astype(np.float32).T)  # [D, K]
    memT = _k_mid(memT.astype(ml_dtypes.bfloat16))
    eyen = (np.eye(128, dtype=np.float32) * 1e30)

    in_maps = []
    for r in range(NCORES):
        zr = np.roll(zT, -APC * r, axis=1)              # own cols first
        anch = (zr[:, :APC] * np.float32(INV_T)).astype(ml_dtypes.bfloat16)
        in_maps.append({
            "anch": _k_mid(anch),
            "zrot": _k_mid(zr.astype(ml_dtypes.bfloat16)),
            "memq": memT,
            "eyen": eyen,
        })
    return in_maps


def combine_outputs(results, z: np.ndarray) -> np.ndarray:
    # results[r]["stats_out"]: [128, 2, NB*NCH]; anchor g = 512r+128b+p.
    # lse = M + log(sum_c S_c * exp(m_c - M)),  m_c = -nm_c, M = max_c m_c
    lse_all = np.empty(N, dtype=np.float64)
    for r in range(NCORES):
        stats = np.asarray(results[r]["stats_out"], dtype=np.float64)
        m = -stats[:, 0, :].reshape(128, NB, NCH)
        S = stats[:, 1, :].reshape(128, NB, NCH)
        M = m.max(axis=2)                                        # [128, NB]
        lse = M + np.log(np.sum(S * np.exp(m - M[:, :, None]), axis=2))
        for b in range(NB):
            g0 = APC * r + 128 * b
            lse_all[g0:g0 + 128] = lse[:, b]
    z64 = z.astype(np.float64)
    pos = np.einsum("ij,ij->i", z64[:-1], z64[1:]) * INV_T       # [N-1]
    pp = lse_all[:N - 1] - pos
    valid = (np.arange(N - 1) % L) != (L - 1)
    loss = pp[valid].sum() / valid.sum()
    return np.float32(loss)


def kernel(z_t, va_values=None, memory_queue=None, _trace=False):
    nc = _get_nc()
    in_maps = make_in_maps(z_t, memory_queue)
    res = run_bass_kernel_spmd(
        nc, in_maps, core_ids=list(range(NCORES)), trace=_trace,
    )
    out = combine_outputs(res.results, z_t.reshape(N, D))
    if _trace:
        kernel.last_result = res
    return out


if __name__ == "__main__":
    rng = np.random.default_rng(0)
    z_t = rng.standard_normal((B, L, D), dtype=np.float32)
    mq = rng.standard_normal((K, D), dtype=np.float32)
    va = rng.random((B, L, 2), dtype=np.float32)
    loss = kernel(z_t, va, mq)
    print("device loss:", loss)
    # numpy reference check
    z = z_t.reshape(N, D).astype(np.float64)
    sim = (z @ z.T) * INV_T
    msim = (z @ mq.astype(np.float64).T) * INV_T
    np.fill_diagonal(sim, -np.inf)
    logits = np.concatenate([sim, msim], axis=1)
    m = logits.max(axis=1, keepdims=True)
    lse = np.log(np.exp(logits - m).sum(axis=1)) + m[:, 0]
    pos = np.array([(z[i] @ z[i + 1]) * INV_T for i in range(N - 1)])
    ppz = -pos + lse[:-1]
    vald = (np.arange(N - 1) % L) != (L - 1)
    ref = ppz[vald].sum() / vald.sum()
    print("numpy  loss:", ref, " rel err:", abs(loss - ref) / abs(ref))
